# revision 40
# baseline (speedup 1.0000x reference)
"""
Trainium2 Bass kernel for batched cross-attention:
  context[b] = softmax(q[b] @ tokens[b].T / sqrt(d)) @ tokens[b]
with x_latent (tokens) [16, 4096, 768] f32, prompts_latent (q) [16, 64, 768] f32.

Sharding: data-parallel over batch — 16 batches / 8 cores = 2 per core.

Final design (measured 76us at full PE clock, 87us when the part throttles to
~2.0GHz; baseline two-layout DMA-bound kernel: 92-108us):

Tokens ship in d-major layout tt [768, 4096] bf16 (12.6 MB/core); the n-major
layout needed by the PV matmul is produced ON CHIP by PE transposes for 3 of
every 4 token-tile pairs, and DMA'd from HBM (natural layout, clock-invariant)
for the 4th pair — balancing the PE (~54us busy) against DMA (~46us busy) so
either one throttling degrades gracefully.

Per n-tile t (128 tokens), tt tiles [128d, 128n] are PE stationaries shared by
two instructions:
  - S^T slice [128n, 64p] += tt[c,t]^T @ qt[c]    (6 matmuls, 64-col streams)
  - tn[t] [128n, 768d]     = tt[:,t]^T            (6 PE transposes, 128-col)
  - mm2: O[64,512] += P^T^T tn[:,0:512], O2[64,257] += P^T^T tn[:,512:769] —
    col 256 of O2 accumulates the softmax row sums through a pre-seeded ones
    column in the SBUF tn tiles (no separate sum/reduce instructions).

Phase structure per batch: Phase 1 emits all transpose+S^T tiles (wait-light,
runs at ~510ns/tile streams); Phase 2 emits all mm2 pairs, whose semaphore
checks hide under their 213/107-col streams; the scheduler then interleaves
them. Rationale: every semaphore check costs ~100ns of PE sequencer stall and
starves the weight-load pipe for ~450ns when sandwiched between short streams.

Dependency granularity is kept coarse: ONE tt DMA per 8-tile group (with small
2/2/4-tile ramp-up groups so the PE starts ~3us in), ONE ACT exp per 4-tile
half-group ([128,256] S^T slices packed in one PSUM bank), ONE DVE copy per
2-tile pair (psum [128,1536] -> strided SBUF [128,1538]). The identity matrix
ships from the host (make_identity needs the GpSimd library load, which
otherwise delays the first transpose by ~7us).
"""

import os
import sys

import numpy as np

for _p in ("/opt/trn_rl_repo", "/root/.axon_site/_ro/trn_rl_repo"):
    if os.path.isdir(_p) and _p not in sys.path:
        sys.path.append(_p)

import ml_dtypes
from contextlib import ExitStack

import concourse.bass as bass
import concourse.mybir as mybir
import concourse.tile as tile
from concourse import bacc
from concourse.bass_utils import run_bass_kernel_spmd

BF16 = ml_dtypes.bfloat16

N_CORES = 8
B_TOTAL = 16
BPC = B_TOTAL // N_CORES
N = 4096
D = 768
P = 64
DC = D // 128   # 6 d-chunks
NT = N // 128   # 32 n-tiles per batch
GPT = 8         # n-tiles per DMA/exp group
NG = NT // GPT  # 4 groups per batch
HG = 4          # tiles per exp half-group
SCALE = float(D) ** -0.5
TN_BUFS = 18
TT_BUFS = 4
# DMA group sizes (in n-tiles): small ramp-up groups so the PE can start
# ~3.5us in instead of waiting out a 1.5MB first load.
GROUPS_B0 = [2, 2, 4, 8, 8, 8]
GROUPS_BN = [8, 8, 8, 8]

_cached_nc = None


def build_bass_program() -> bass.Bass:
    nc = bacc.Bacc("TRN2", target_bir_lowering=False, debug=False)
    qt = nc.declare_dram_parameter("qt", [BPC, D, P], mybir.dt.bfloat16, isOutput=False)
    tt = nc.declare_dram_parameter("tt", [BPC, D, N], mybir.dt.bfloat16, isOutput=False)
    tn = nc.declare_dram_parameter("tn", [BPC, N, D], mybir.dt.bfloat16, isOutput=False)
    idm = nc.declare_dram_parameter("idm", [128, 128], mybir.dt.bfloat16, isOutput=False)
    out = nc.declare_dram_parameter("out", [BPC, P, D], mybir.dt.float32, isOutput=True)

    with tile.TileContext(nc) as tc, ExitStack() as ctx:
        singles = ctx.enter_context(tc.tile_pool(name="singles", bufs=1))
        qt_pool = ctx.enter_context(tc.tile_pool(name="qtp", bufs=2))
        tt_pool = ctx.enter_context(tc.tile_pool(name="ttp", bufs=TT_BUFS))
        tn_pool = ctx.enter_context(tc.tile_pool(name="tnp", bufs=TN_BUFS))
        pt_pool = ctx.enter_context(tc.tile_pool(name="ptp", bufs=9))
        o_pool = ctx.enter_context(tc.tile_pool(name="op", bufs=2))
        fin_pool = ctx.enter_context(tc.tile_pool(name="finp", bufs=2))
        ps = ctx.enter_context(tc.tile_pool(name="ps", bufs=2, space="PSUM"))

        ident = singles.tile([128, 128], mybir.dt.bfloat16)

        qt_ts = [None] * BPC
        o_ab = {}           # b -> (o_a, o_b2)
        group_tiles = {}    # (b, g) -> tt_g tile

        def load_qt(b):
            qt_ts[b] = qt_pool.tile([128, DC, P], mybir.dt.bfloat16, tag="qt", name="qt_t")
            nc.sync.dma_start(out=qt_ts[b], in_=qt[b].rearrange("(c p) m -> p c m", p=128))

        # flat group list: (b, start_tile, n_tiles)
        flat_groups = []
        for b in range(BPC):
            sizes = GROUPS_B0 if b == 0 else GROUPS_BN
            s = 0
            for n in sizes:
                flat_groups.append((b, s, n))
                s += n
        tile_group = {}   # (b, t) -> (group_index, offset_in_group)
        for gi, (b, s, n) in enumerate(flat_groups):
            for k in range(n):
                tile_group[(b, s + k)] = (gi, k)

        def load_group(gi):
            b, s, n = flat_groups[gi]
            tt_g = tt_pool.tile([128, DC, n * 128], mybir.dt.bfloat16, tag="ttg",
                                name="tt_g", padded_shape=[128, DC, GPT * 128])
            tt_r = tt[b].rearrange("(c p) n -> p c n", p=128)
            if gi < 2:
                # head groups: per-chunk DMAs so tile 0's first transposes can
                # start on the first 64KB instead of the whole group
                for c in range(DC):
                    nc.sync.dma_start(out=tt_g[:, c, :],
                                      in_=tt_r[:, c, s * 128:(s + n) * 128])
            else:
                nc.sync.dma_start(out=tt_g, in_=tt_r[:, :, s * 128:(s + n) * 128])
            group_tiles[gi] = tt_g

        def ensure_o(b):
            if b not in o_ab:
                o_a = ps.tile([P, 512], mybir.dt.float32, tag="o_a", bufs=1, name="o_a")
                o_b2 = ps.tile([P, 257], mybir.dt.float32, tag="o_b", bufs=1, name="o_b2")
                o_ab[b] = (o_a, o_b2)

        def mm2a(b2, t2, pt2, tn2, half):
            ensure_o(b2)
            o_a, _ = o_ab[b2]
            base = half * (D + 1)
            nc.tensor.matmul(o_a, lhsT=pt2, rhs=tn2[:, base:base + 512],
                             start=(t2 == 0), stop=(t2 == NT - 1))

        def mm2b(b2, t2, pt2, tn2, half):
            _, o_b2 = o_ab[b2]
            base = half * (D + 1)
            nc.tensor.matmul(o_b2, lhsT=pt2, rhs=tn2[:, base + 512:base + D + 1],
                             start=(t2 == 0), stop=(t2 == NT - 1))
            if t2 == NT - 1:
                finalize(b2)

        def finalize(b):
            o_a, o_b2 = o_ab[b]
            rec = fin_pool.tile([P, 1], mybir.dt.float32, tag="rec", name="rec")
            nc.vector.reciprocal(rec, o_b2[:, 256:257])
            o_sb = o_pool.tile([P, D], mybir.dt.float32, tag="osb", name="o_sb")
            # split the normalization across DVE and ACT so they run in
            # parallel, and ship each half as soon as it is scaled
            nc.vector.tensor_scalar_mul(o_sb[:, 0:512], o_a, rec)
            nc.sync.dma_start(out=out[b][:, 0:512], in_=o_sb[:, 0:512])
            nc.scalar.activation(out=o_sb[:, 512:D], in_=o_b2[:, 0:256],
                                 func=mybir.ActivationFunctionType.Copy, scale=rec)
            nc.sync.dma_start(out=out[b][:, 512:D], in_=o_sb[:, 512:D])
            del o_ab[b]

        # prologue: ident, qt, first two tt groups; ring-slot seeding after.
        nc.sync.dma_start(out=ident, in_=idm[:, :])
        load_qt(0)
        load_group(0)
        load_group(1)
        # PE p-state warmup while the head DMAs stream in: ~30 dummy matmuls
        # keep the PE continuously busy so real tiles start at 2.4GHz
        warm = ps.tile([128, 512], mybir.dt.float32, tag="st", name="warm")
        for _ in range(30):
            nc.tensor.matmul(warm[:, 0:128], lhsT=ident, rhs=ident, start=True, stop=True)
        # Pre-seed the ones columns (768 and 1537) of every tn ring slot once.
        for _ in range(TN_BUFS):
            t0 = tn_pool.tile([128, 2 * D + 2], mybir.dt.bfloat16, tag="tn", name="tn_seed")
            nc.vector.memset(t0[:, D:D + 1], 1.0)
            nc.vector.memset(t0[:, 2 * D + 1:2 * D + 2], 1.0)

        mm2_q = []        # per-tile mm2 descriptors (b, t, pt_slice, tn_sb, half)
        st_g = None
        tn_ps = None
        tn_halves = []    # tn_sb tiles of the current half-group
        half_accum = []   # (b, t) of tiles in current half-group
        pair_tiles = {}   # pair index -> tn_sb tile (DMA'd pairs arrive early)
        # issue each DMA'd pair's transfer 4 pairs ahead of its tiles
        tn_issue = {}
        for k in range(NT // 2):
            if k % 4 == 3:
                tn_issue.setdefault(max(0, k - 4), []).append(k)

        for b in range(BPC):
            # Phase 1: all transposes + S^T matmuls for batch b (wait-light).
            tn_r = tn[b].rearrange("(t p) d -> p t d", p=128)
            for t in range(NT):
                g, j = divmod(t, GPT)
                gi, goff = tile_group[(b, t)]
                glen = flat_groups[gi][2]
                # prefetch at the END of each group so the head groups' DMAs
                # aren't queued behind later groups' bulk transfers
                if goff == glen - 1 and gi + 2 < len(flat_groups):
                    load_group(gi + 2)
                if t == NT - GPT and b + 1 < BPC:
                    load_qt(b + 1)
                if j == 0:
                    st_g = ps.tile([128, GPT * P], mybir.dt.float32, tag="st", name="st_g")
                # 3 of every 8 token pairs ship from HBM in natural layout
                # (clock-invariant DMA) instead of a PE transpose; DMAs are
                # issued two pairs ahead for delivery slack
                p = t // 2
                dma_pair = p % 4 == 3
                if j % 2 == 0:
                    for k in tn_issue.get(p, ()):
                        tn_nx = tn_pool.tile([128, 2 * D + 2], mybir.dt.bfloat16, tag="tn", name="tn_nx")
                        nc.sync.dma_start(
                            out=tn_nx.rearrange("p (k x) -> p k x", k=2)[:, :, 0:D],
                            in_=tn_r[:, 2 * k:2 * k + 2, :],
                        )
                        pair_tiles[k] = tn_nx
                if j % 2 == 0 and not dma_pair:
                    tn_ps = ps.tile([128, 2 * D], mybir.dt.bfloat16, tag="tnps", name="tn_ps")
                tt_g = group_tiles[gi]
                qt_t = qt_ts[b]
                half = j % 2

                for c in range(DC):
                    stat = tt_g[:, c, goff * 128:(goff + 1) * 128]
                    if not dma_pair:
                        nc.tensor.transpose(
                            tn_ps[:, half * D + c * 128:half * D + (c + 1) * 128], stat, ident)
                    nc.tensor.matmul(st_g[:, j * P:(j + 1) * P], lhsT=stat, rhs=qt_t[:, c, :],
                                     start=(c == 0), stop=(c == DC - 1))
                half_accum.append((b, t))

                if half == 1 and not dma_pair:
                    tn_sb = tn_pool.tile([128, 2 * D + 2], mybir.dt.bfloat16, tag="tn", name="tn_sb")
                    nc.vector.tensor_copy(
                        tn_sb.rearrange("p (k x) -> p k x", k=2)[:, :, 0:D],
                        tn_ps.rearrange("p (k x) -> p k x", k=2),
                    )
                    pair_tiles[p] = tn_sb
                if half == 1:
                    tn_halves.append(pair_tiles.pop(p))

                if j % HG == HG - 1:
                    # half-group complete: one exp for 4 tiles
                    h = (j // HG) % 2
                    pt_h = pt_pool.tile([128, HG * P], mybir.dt.bfloat16, tag="pt", name="pt_h")
                    nc.scalar.activation(out=pt_h, in_=st_g[:, h * HG * P:(h + 1) * HG * P],
                                         func=mybir.ActivationFunctionType.Exp, scale=SCALE)
                    for k, (b2, t2) in enumerate(half_accum):
                        mm2_q.append((b2, t2, pt_h[:, k * P:(k + 1) * P],
                                      tn_halves[k // 2], k % 2))
                    tn_halves = []
                    half_accum = []

            # Phase 2: all mm2 pairs back-to-back; their semaphore checks hide
            # under the 213/107-col streams (measured clean at ~330ns/pair).
            while mm2_q:
                mm2 = mm2_q.pop(0)
                mm2a(*mm2)
                mm2b(*mm2)

    nc.compile()
    return nc


def _get_nc() -> bass.Bass:
    global _cached_nc
    if _cached_nc is None:
        _cached_nc = build_bass_program()
    return _cached_nc


def _make_in_maps(x_latent: np.ndarray, prompts_latent: np.ndarray):
    tn_h = np.ascontiguousarray(x_latent.astype(BF16))             # [16, N, D]
    tt_h = np.ascontiguousarray(tn_h.transpose(0, 2, 1))           # [16, D, N]
    qt_h = np.ascontiguousarray(prompts_latent.astype(BF16).transpose(0, 2, 1))
    idm = np.eye(128, dtype=BF16)
    return [
        {
            "qt": qt_h[c * BPC:(c + 1) * BPC],
            "tt": tt_h[c * BPC:(c + 1) * BPC],
            "tn": tn_h[c * BPC:(c + 1) * BPC],
            "idm": idm,
        }
        for c in range(N_CORES)
    ]


def run(x_latent: np.ndarray, prompts_latent: np.ndarray, trace: bool = False):
    """Run on all 8 cores; returns (output [16, 64, 768] f32, BassKernelResults)."""
    nc = _get_nc()
    in_maps = _make_in_maps(np.asarray(x_latent), np.asarray(prompts_latent))
    res = run_bass_kernel_spmd(nc, in_maps, list(range(N_CORES)), trace=trace)
    out = np.concatenate([np.asarray(r["out"]) for r in res.results], axis=0)
    return out.astype(np.float32), res


def kernel(x_latent: np.ndarray, prompts_latent: np.ndarray) -> np.ndarray:
    out, _ = run(x_latent, prompts_latent, trace=False)
    return out


# revision 43
# speedup vs baseline: 1.1114x; 1.1114x over previous
"""
Trainium2 Bass kernel for batched cross-attention:
  context[b] = softmax(q[b] @ tokens[b].T / sqrt(d)) @ tokens[b]
with x_latent (tokens) [16, 4096, 768] f32, prompts_latent (q) [16, 64, 768] f32.

Sharding: data-parallel over batch — 16 batches / 8 cores = 2 per core.

Final design (measured 76us at full PE clock, 87us when the part throttles to
~2.0GHz; baseline two-layout DMA-bound kernel: 92-108us):

Tokens ship in d-major layout tt [768, 4096] bf16 (12.6 MB/core); the n-major
layout needed by the PV matmul is produced ON CHIP by PE transposes for 3 of
every 4 token-tile pairs, and DMA'd from HBM (natural layout, clock-invariant)
for the 4th pair — balancing the PE (~54us busy) against DMA (~46us busy) so
either one throttling degrades gracefully.

Per n-tile t (128 tokens), tt tiles [128d, 128n] are PE stationaries shared by
two instructions:
  - S^T slice [128n, 64p] += tt[c,t]^T @ qt[c]    (6 matmuls, 64-col streams)
  - tn[t] [128n, 768d]     = tt[:,t]^T            (6 PE transposes, 128-col)
  - mm2: O[64,512] += P^T^T tn[:,0:512], O2[64,257] += P^T^T tn[:,512:769] —
    col 256 of O2 accumulates the softmax row sums through a pre-seeded ones
    column in the SBUF tn tiles (no separate sum/reduce instructions).

Phase structure per batch: Phase 1 emits all transpose+S^T tiles (wait-light,
runs at ~510ns/tile streams); Phase 2 emits all mm2 pairs, whose semaphore
checks hide under their 213/107-col streams; the scheduler then interleaves
them. Rationale: every semaphore check costs ~100ns of PE sequencer stall and
starves the weight-load pipe for ~450ns when sandwiched between short streams.

Dependency granularity is kept coarse: ONE tt DMA per 8-tile group (with small
2/2/4-tile ramp-up groups so the PE starts ~3us in), ONE ACT exp per 4-tile
half-group ([128,256] S^T slices packed in one PSUM bank), ONE DVE copy per
2-tile pair (psum [128,1536] -> strided SBUF [128,1538]). The identity matrix
ships from the host (make_identity needs the GpSimd library load, which
otherwise delays the first transpose by ~7us).
"""

import os
import sys

import numpy as np

for _p in ("/opt/trn_rl_repo", "/root/.axon_site/_ro/trn_rl_repo"):
    if os.path.isdir(_p) and _p not in sys.path:
        sys.path.append(_p)

import ml_dtypes
from contextlib import ExitStack

import concourse.bass as bass
import concourse.mybir as mybir
import concourse.tile as tile
from concourse import bacc
from concourse.bass_utils import run_bass_kernel_spmd

BF16 = ml_dtypes.bfloat16

N_CORES = 8
B_TOTAL = 16
BPC = B_TOTAL // N_CORES
N = 4096
D = 768
P = 64
DC = D // 128   # 6 d-chunks
NT = N // 128   # 32 n-tiles per batch
GPT = 8         # n-tiles per DMA/exp group
NG = NT // GPT  # 4 groups per batch
HG = 4          # tiles per exp half-group
SCALE = float(D) ** -0.5
TN_BUFS = 18
TT_BUFS = 4
# DMA group sizes (in n-tiles): small ramp-up groups so the PE can start
# ~3.5us in instead of waiting out a 1.5MB first load.
GROUPS_B0 = [2, 2, 4, 8, 8, 8]
GROUPS_BN = [8, 8, 8, 8]

_cached_nc = None


def build_bass_program() -> bass.Bass:
    nc = bacc.Bacc("TRN2", target_bir_lowering=False, debug=False)
    qt = nc.declare_dram_parameter("qt", [BPC, D, P], mybir.dt.bfloat16, isOutput=False)
    tt = nc.declare_dram_parameter("tt", [BPC, D, N], mybir.dt.bfloat16, isOutput=False)
    tn = nc.declare_dram_parameter("tn", [BPC, N, D], mybir.dt.bfloat16, isOutput=False)
    idm = nc.declare_dram_parameter("idm", [128, 128], mybir.dt.bfloat16, isOutput=False)
    out = nc.declare_dram_parameter("out", [BPC, P, D], mybir.dt.float32, isOutput=True)

    with tile.TileContext(nc) as tc, ExitStack() as ctx:
        singles = ctx.enter_context(tc.tile_pool(name="singles", bufs=1))
        qt_pool = ctx.enter_context(tc.tile_pool(name="qtp", bufs=2))
        tt_pool = ctx.enter_context(tc.tile_pool(name="ttp", bufs=TT_BUFS))
        tn_pool = ctx.enter_context(tc.tile_pool(name="tnp", bufs=TN_BUFS))
        pt_pool = ctx.enter_context(tc.tile_pool(name="ptp", bufs=9))
        o_pool = ctx.enter_context(tc.tile_pool(name="op", bufs=2))
        fin_pool = ctx.enter_context(tc.tile_pool(name="finp", bufs=2))
        ps = ctx.enter_context(tc.tile_pool(name="ps", bufs=2, space="PSUM"))

        ident = singles.tile([128, 128], mybir.dt.bfloat16)

        qt_ts = [None] * BPC
        o_ab = {}           # b -> (o_a, o_b2)
        group_tiles = {}    # (b, g) -> tt_g tile

        def load_qt(b):
            qt_ts[b] = qt_pool.tile([128, DC, P], mybir.dt.bfloat16, tag="qt", name="qt_t")
            nc.sync.dma_start(out=qt_ts[b], in_=qt[b].rearrange("(c p) m -> p c m", p=128))

        # flat group list: (b, start_tile, n_tiles)
        flat_groups = []
        for b in range(BPC):
            sizes = GROUPS_B0 if b == 0 else GROUPS_BN
            s = 0
            for n in sizes:
                flat_groups.append((b, s, n))
                s += n
        tile_group = {}   # (b, t) -> (group_index, offset_in_group)
        for gi, (b, s, n) in enumerate(flat_groups):
            for k in range(n):
                tile_group[(b, s + k)] = (gi, k)

        def load_group(gi):
            b, s, n = flat_groups[gi]
            tt_g = tt_pool.tile([128, DC, n * 128], mybir.dt.bfloat16, tag="ttg",
                                name="tt_g", padded_shape=[128, DC, GPT * 128])
            tt_r = tt[b].rearrange("(c p) n -> p c n", p=128)
            nc.sync.dma_start(out=tt_g, in_=tt_r[:, :, s * 128:(s + n) * 128])
            group_tiles[gi] = tt_g

        def ensure_o(b):
            if b not in o_ab:
                o_a = ps.tile([P, 512], mybir.dt.float32, tag="o_a", bufs=1, name="o_a")
                o_b2 = ps.tile([P, 257], mybir.dt.float32, tag="o_b", bufs=1, name="o_b2")
                o_ab[b] = (o_a, o_b2)

        def mm2a(b2, t2, pt2, tn2, half):
            ensure_o(b2)
            o_a, _ = o_ab[b2]
            base = half * (D + 1)
            nc.tensor.matmul(o_a, lhsT=pt2, rhs=tn2[:, base:base + 512],
                             start=(t2 == 0), stop=(t2 == NT - 1))

        def mm2b(b2, t2, pt2, tn2, half):
            _, o_b2 = o_ab[b2]
            base = half * (D + 1)
            nc.tensor.matmul(o_b2, lhsT=pt2, rhs=tn2[:, base + 512:base + D + 1],
                             start=(t2 == 0), stop=(t2 == NT - 1))
            if t2 == NT - 1:
                finalize(b2)

        def finalize(b):
            o_a, o_b2 = o_ab[b]
            rec = fin_pool.tile([P, 1], mybir.dt.float32, tag="rec", name="rec")
            nc.vector.reciprocal(rec, o_b2[:, 256:257])
            o_sb = o_pool.tile([P, D], mybir.dt.float32, tag="osb", name="o_sb")
            # split the normalization across DVE and ACT so they run in
            # parallel, and ship each half as soon as it is scaled
            nc.vector.tensor_scalar_mul(o_sb[:, 0:512], o_a, rec)
            nc.sync.dma_start(out=out[b][:, 0:512], in_=o_sb[:, 0:512])
            nc.scalar.activation(out=o_sb[:, 512:D], in_=o_b2[:, 0:256],
                                 func=mybir.ActivationFunctionType.Copy, scale=rec)
            nc.sync.dma_start(out=out[b][:, 512:D], in_=o_sb[:, 512:D])
            del o_ab[b]

        # prologue: ident, qt, first two tt groups; ring-slot seeding after.
        nc.sync.dma_start(out=ident, in_=idm[:, :])
        load_qt(0)
        load_group(0)
        load_group(1)
        # Pre-seed the ones columns (768 and 1537) of every tn ring slot once.
        for _ in range(TN_BUFS):
            t0 = tn_pool.tile([128, 2 * D + 2], mybir.dt.bfloat16, tag="tn", name="tn_seed")
            nc.vector.memset(t0[:, D:D + 1], 1.0)
            nc.vector.memset(t0[:, 2 * D + 1:2 * D + 2], 1.0)

        mm2_q = []        # per-tile mm2 descriptors (b, t, pt_slice, tn_sb, half)
        st_g = None
        tn_ps = None
        tn_halves = []    # tn_sb tiles of the current half-group
        half_accum = []   # (b, t) of tiles in current half-group
        pair_tiles = {}   # pair index -> tn_sb tile (DMA'd pairs arrive early)
        # issue each DMA'd pair's transfer 4 pairs ahead of its tiles
        tn_issue = {}
        for k in range(NT // 2):
            if k % 4 == 3:
                tn_issue.setdefault(max(0, k - 1), []).append(k)

        for b in range(BPC):
            # Phase 1: all transposes + S^T matmuls for batch b (wait-light).
            tn_r = tn[b].rearrange("(t p) d -> p t d", p=128)
            for t in range(NT):
                g, j = divmod(t, GPT)
                gi, goff = tile_group[(b, t)]
                glen = flat_groups[gi][2]
                # prefetch at the END of each group so the head groups' DMAs
                # aren't queued behind later groups' bulk transfers
                if goff == glen - 1 and gi + 2 < len(flat_groups):
                    load_group(gi + 2)
                if t == NT - GPT and b + 1 < BPC:
                    load_qt(b + 1)
                if j == 0:
                    st_g = ps.tile([128, GPT * P], mybir.dt.float32, tag="st", name="st_g")
                # 3 of every 8 token pairs ship from HBM in natural layout
                # (clock-invariant DMA) instead of a PE transpose; DMAs are
                # issued two pairs ahead for delivery slack
                p = t // 2
                dma_pair = p % 4 == 3
                if j % 2 == 0:
                    for k in tn_issue.get(p, ()):
                        tn_nx = tn_pool.tile([128, 2 * D + 2], mybir.dt.bfloat16, tag="tn", name="tn_nx")
                        nc.sync.dma_start(
                            out=tn_nx.rearrange("p (k x) -> p k x", k=2)[:, :, 0:D],
                            in_=tn_r[:, 2 * k:2 * k + 2, :],
                        )
                        pair_tiles[k] = tn_nx
                if j % 2 == 0 and not dma_pair:
                    tn_ps = ps.tile([128, 2 * D], mybir.dt.bfloat16, tag="tnps", name="tn_ps")
                tt_g = group_tiles[gi]
                qt_t = qt_ts[b]
                half = j % 2

                for c in range(DC):
                    stat = tt_g[:, c, goff * 128:(goff + 1) * 128]
                    if not dma_pair:
                        nc.tensor.transpose(
                            tn_ps[:, half * D + c * 128:half * D + (c + 1) * 128], stat, ident)
                    nc.tensor.matmul(st_g[:, j * P:(j + 1) * P], lhsT=stat, rhs=qt_t[:, c, :],
                                     start=(c == 0), stop=(c == DC - 1))
                half_accum.append((b, t))

                if half == 1 and not dma_pair:
                    tn_sb = tn_pool.tile([128, 2 * D + 2], mybir.dt.bfloat16, tag="tn", name="tn_sb")
                    nc.vector.tensor_copy(
                        tn_sb.rearrange("p (k x) -> p k x", k=2)[:, :, 0:D],
                        tn_ps.rearrange("p (k x) -> p k x", k=2),
                    )
                    pair_tiles[p] = tn_sb
                if half == 1:
                    tn_halves.append(pair_tiles.pop(p))

                if j % HG == HG - 1:
                    # half-group complete: one exp for 4 tiles
                    h = (j // HG) % 2
                    pt_h = pt_pool.tile([128, HG * P], mybir.dt.bfloat16, tag="pt", name="pt_h")
                    nc.scalar.activation(out=pt_h, in_=st_g[:, h * HG * P:(h + 1) * HG * P],
                                         func=mybir.ActivationFunctionType.Exp, scale=SCALE)
                    for k, (b2, t2) in enumerate(half_accum):
                        mm2_q.append((b2, t2, pt_h[:, k * P:(k + 1) * P],
                                      tn_halves[k // 2], k % 2))
                    tn_halves = []
                    half_accum = []

            # Phase 2: all mm2 pairs back-to-back; their semaphore checks hide
            # under the 213/107-col streams (measured clean at ~330ns/pair).
            while mm2_q:
                mm2 = mm2_q.pop(0)
                mm2a(*mm2)
                mm2b(*mm2)

    nc.compile()
    return nc


def _get_nc() -> bass.Bass:
    global _cached_nc
    if _cached_nc is None:
        _cached_nc = build_bass_program()
    return _cached_nc


def _make_in_maps(x_latent: np.ndarray, prompts_latent: np.ndarray):
    tn_h = np.ascontiguousarray(x_latent.astype(BF16))             # [16, N, D]
    tt_h = np.ascontiguousarray(tn_h.transpose(0, 2, 1))           # [16, D, N]
    qt_h = np.ascontiguousarray(prompts_latent.astype(BF16).transpose(0, 2, 1))
    idm = np.eye(128, dtype=BF16)
    return [
        {
            "qt": qt_h[c * BPC:(c + 1) * BPC],
            "tt": tt_h[c * BPC:(c + 1) * BPC],
            "tn": tn_h[c * BPC:(c + 1) * BPC],
            "idm": idm,
        }
        for c in range(N_CORES)
    ]


def run(x_latent: np.ndarray, prompts_latent: np.ndarray, trace: bool = False):
    """Run on all 8 cores; returns (output [16, 64, 768] f32, BassKernelResults)."""
    nc = _get_nc()
    in_maps = _make_in_maps(np.asarray(x_latent), np.asarray(prompts_latent))
    res = run_bass_kernel_spmd(nc, in_maps, list(range(N_CORES)), trace=trace)
    out = np.concatenate([np.asarray(r["out"]) for r in res.results], axis=0)
    return out.astype(np.float32), res


def kernel(x_latent: np.ndarray, prompts_latent: np.ndarray) -> np.ndarray:
    out, _ = run(x_latent, prompts_latent, trace=False)
    return out


# revision 45
# speedup vs baseline: 1.1846x; 1.0659x over previous
"""
Trainium2 Bass kernel for batched cross-attention:
  context[b] = softmax(q[b] @ tokens[b].T / sqrt(d)) @ tokens[b]
with x_latent (tokens) [16, 4096, 768] f32, prompts_latent (q) [16, 64, 768] f32.

Sharding: data-parallel over batch — 16 batches / 8 cores = 2 per core.

Final design (measured 76us at full PE clock, 87us when the part throttles to
~2.0GHz; baseline two-layout DMA-bound kernel: 92-108us):

Tokens ship in d-major layout tt [768, 4096] bf16 (12.6 MB/core); the n-major
layout needed by the PV matmul is produced ON CHIP by PE transposes for 3 of
every 4 token-tile pairs, and DMA'd from HBM (natural layout, clock-invariant)
for the 4th pair — balancing the PE (~54us busy) against DMA (~46us busy) so
either one throttling degrades gracefully.

Per n-tile t (128 tokens), tt tiles [128d, 128n] are PE stationaries shared by
two instructions:
  - S^T slice [128n, 64p] += tt[c,t]^T @ qt[c]    (6 matmuls, 64-col streams)
  - tn[t] [128n, 768d]     = tt[:,t]^T            (6 PE transposes, 128-col)
  - mm2: O[64,512] += P^T^T tn[:,0:512], O2[64,257] += P^T^T tn[:,512:769] —
    col 256 of O2 accumulates the softmax row sums through a pre-seeded ones
    column in the SBUF tn tiles (no separate sum/reduce instructions).

Phase structure per batch: Phase 1 emits all transpose+S^T tiles (wait-light,
runs at ~510ns/tile streams); Phase 2 emits all mm2 pairs, whose semaphore
checks hide under their 213/107-col streams; the scheduler then interleaves
them. Rationale: every semaphore check costs ~100ns of PE sequencer stall and
starves the weight-load pipe for ~450ns when sandwiched between short streams.

Dependency granularity is kept coarse: ONE tt DMA per 8-tile group (with small
2/2/4-tile ramp-up groups so the PE starts ~3us in), ONE ACT exp per 4-tile
half-group ([128,256] S^T slices packed in one PSUM bank), ONE DVE copy per
2-tile pair (psum [128,1536] -> strided SBUF [128,1538]). The identity matrix
ships from the host (make_identity needs the GpSimd library load, which
otherwise delays the first transpose by ~7us).
"""

import os
import sys

import numpy as np

for _p in ("/opt/trn_rl_repo", "/root/.axon_site/_ro/trn_rl_repo"):
    if os.path.isdir(_p) and _p not in sys.path:
        sys.path.append(_p)

import ml_dtypes
from contextlib import ExitStack

import concourse.bass as bass
import concourse.mybir as mybir
import concourse.tile as tile
from concourse import bacc
from concourse.bass_utils import run_bass_kernel_spmd

BF16 = ml_dtypes.bfloat16

N_CORES = 8
B_TOTAL = 16
BPC = B_TOTAL // N_CORES
N = 4096
D = 768
P = 64
DC = D // 128   # 6 d-chunks
NT = N // 128   # 32 n-tiles per batch
GPT = 8         # n-tiles per DMA/exp group
NG = NT // GPT  # 4 groups per batch
HG = 4          # tiles per exp half-group
SCALE = float(D) ** -0.5
TN_BUFS = 18
TT_BUFS = 4
# DMA group sizes (in n-tiles): small ramp-up groups so the PE can start
# ~3.5us in instead of waiting out a 1.5MB first load.
GROUPS_B0 = [2, 2, 4, 8, 8, 8]
GROUPS_BN = [8, 8, 8, 8]

_cached_nc = None


def build_bass_program() -> bass.Bass:
    nc = bacc.Bacc("TRN2", target_bir_lowering=False, debug=False)
    qt = nc.declare_dram_parameter("qt", [BPC, D, P], mybir.dt.bfloat16, isOutput=False)
    tt = nc.declare_dram_parameter("tt", [BPC, D, N], mybir.dt.bfloat16, isOutput=False)
    tn = nc.declare_dram_parameter("tn", [BPC, N, D], mybir.dt.bfloat16, isOutput=False)
    idm = nc.declare_dram_parameter("idm", [128, 128], mybir.dt.bfloat16, isOutput=False)
    out = nc.declare_dram_parameter("out", [BPC, P, D], mybir.dt.float32, isOutput=True)

    with tile.TileContext(nc) as tc, ExitStack() as ctx:
        singles = ctx.enter_context(tc.tile_pool(name="singles", bufs=1))
        qt_pool = ctx.enter_context(tc.tile_pool(name="qtp", bufs=2))
        tt_pool = ctx.enter_context(tc.tile_pool(name="ttp", bufs=TT_BUFS))
        tn_pool = ctx.enter_context(tc.tile_pool(name="tnp", bufs=TN_BUFS))
        pt_pool = ctx.enter_context(tc.tile_pool(name="ptp", bufs=9))
        o_pool = ctx.enter_context(tc.tile_pool(name="op", bufs=2))
        fin_pool = ctx.enter_context(tc.tile_pool(name="finp", bufs=2))
        ps = ctx.enter_context(tc.tile_pool(name="ps", bufs=2, space="PSUM"))

        ident = singles.tile([128, 128], mybir.dt.bfloat16)

        qt_ts = [None] * BPC
        o_ab = {}           # b -> (o_a, o_b2)
        group_tiles = {}    # (b, g) -> tt_g tile

        def load_qt(b):
            qt_ts[b] = qt_pool.tile([128, DC, P], mybir.dt.bfloat16, tag="qt", name="qt_t")
            nc.sync.dma_start(out=qt_ts[b], in_=qt[b].rearrange("(c p) m -> p c m", p=128))

        # flat group list: (b, start_tile, n_tiles)
        flat_groups = []
        for b in range(BPC):
            sizes = GROUPS_B0 if b == 0 else GROUPS_BN
            s = 0
            for n in sizes:
                flat_groups.append((b, s, n))
                s += n
        tile_group = {}   # (b, t) -> (group_index, offset_in_group)
        for gi, (b, s, n) in enumerate(flat_groups):
            for k in range(n):
                tile_group[(b, s + k)] = (gi, k)

        def load_group(gi):
            b, s, n = flat_groups[gi]
            tt_g = tt_pool.tile([128, DC, n * 128], mybir.dt.bfloat16, tag="ttg",
                                name="tt_g", padded_shape=[128, DC, GPT * 128])
            tt_r = tt[b].rearrange("(c p) n -> p c n", p=128)
            if gi < 2:
                # head groups: per-chunk DMAs so tile 0 starts on ~64KB
                for c in range(DC):
                    nc.sync.dma_start(out=tt_g[:, c, :],
                                      in_=tt_r[:, c, s * 128:(s + n) * 128])
            else:
                nc.sync.dma_start(out=tt_g, in_=tt_r[:, :, s * 128:(s + n) * 128])
            group_tiles[gi] = tt_g

        def ensure_o(b):
            if b not in o_ab:
                o_a = ps.tile([P, 512], mybir.dt.float32, tag="o_a", bufs=1, name="o_a")
                o_b2 = ps.tile([P, 257], mybir.dt.float32, tag="o_b", bufs=1, name="o_b2")
                o_ab[b] = (o_a, o_b2)

        def mm2a(b2, t2, pt2, tn2, half):
            ensure_o(b2)
            o_a, _ = o_ab[b2]
            base = half * (D + 1)
            nc.tensor.matmul(o_a, lhsT=pt2, rhs=tn2[:, base:base + 512],
                             start=(t2 == 0), stop=(t2 == NT - 1))

        def mm2b(b2, t2, pt2, tn2, half):
            _, o_b2 = o_ab[b2]
            base = half * (D + 1)
            nc.tensor.matmul(o_b2, lhsT=pt2, rhs=tn2[:, base + 512:base + D + 1],
                             start=(t2 == 0), stop=(t2 == NT - 1))
            if t2 == NT - 1:
                finalize(b2)

        def finalize(b):
            o_a, o_b2 = o_ab[b]
            rec = fin_pool.tile([P, 1], mybir.dt.float32, tag="rec", name="rec")
            nc.vector.reciprocal(rec, o_b2[:, 256:257])
            o_sb = o_pool.tile([P, D], mybir.dt.float32, tag="osb", name="o_sb")
            # split the normalization across DVE and ACT so they run in
            # parallel, and ship each half as soon as it is scaled
            nc.vector.tensor_scalar_mul(o_sb[:, 0:512], o_a, rec)
            nc.sync.dma_start(out=out[b][:, 0:512], in_=o_sb[:, 0:512])
            nc.scalar.activation(out=o_sb[:, 512:D], in_=o_b2[:, 0:256],
                                 func=mybir.ActivationFunctionType.Copy, scale=rec)
            nc.sync.dma_start(out=out[b][:, 512:D], in_=o_sb[:, 512:D])
            del o_ab[b]

        # prologue: ident, qt, first two tt groups; ring-slot seeding after.
        nc.sync.dma_start(out=ident, in_=idm[:, :])
        load_qt(0)
        load_group(0)
        load_group(1)
        # Pre-seed the ones columns (768 and 1537) of every tn ring slot once.
        for _ in range(TN_BUFS):
            t0 = tn_pool.tile([128, 2 * D + 2], mybir.dt.bfloat16, tag="tn", name="tn_seed")
            nc.vector.memset(t0[:, D:D + 1], 1.0)
            nc.vector.memset(t0[:, 2 * D + 1:2 * D + 2], 1.0)

        mm2_q = []        # per-tile mm2 descriptors (b, t, pt_slice, tn_sb, half)
        st_g = None
        tn_ps = None
        tn_halves = []    # tn_sb tiles of the current half-group
        half_accum = []   # (b, t) of tiles in current half-group
        pair_tiles = {}   # pair index -> tn_sb tile (DMA'd pairs arrive early)
        # issue each DMA'd pair's transfer 4 pairs ahead of its tiles
        tn_issue = {}
        for k in range(NT // 2):
            if k % 4 == 3:
                tn_issue.setdefault(max(0, k - 1), []).append(k)

        for b in range(BPC):
            # Phase 1: all transposes + S^T matmuls for batch b (wait-light).
            tn_r = tn[b].rearrange("(t p) d -> p t d", p=128)
            for t in range(NT):
                g, j = divmod(t, GPT)
                gi, goff = tile_group[(b, t)]
                glen = flat_groups[gi][2]
                # prefetch at the END of each group so the head groups' DMAs
                # aren't queued behind later groups' bulk transfers
                if goff == glen - 1 and gi + 2 < len(flat_groups):
                    load_group(gi + 2)
                if t == NT - GPT and b + 1 < BPC:
                    load_qt(b + 1)
                if j == 0:
                    st_g = ps.tile([128, GPT * P], mybir.dt.float32, tag="st", name="st_g")
                # 3 of every 8 token pairs ship from HBM in natural layout
                # (clock-invariant DMA) instead of a PE transpose; DMAs are
                # issued two pairs ahead for delivery slack
                p = t // 2
                dma_pair = p % 4 == 3
                if j % 2 == 0:
                    for k in tn_issue.get(p, ()):
                        tn_nx = tn_pool.tile([128, 2 * D + 2], mybir.dt.bfloat16, tag="tn", name="tn_nx")
                        nc.sync.dma_start(
                            out=tn_nx.rearrange("p (k x) -> p k x", k=2)[:, :, 0:D],
                            in_=tn_r[:, 2 * k:2 * k + 2, :],
                        )
                        pair_tiles[k] = tn_nx
                if j % 2 == 0 and not dma_pair:
                    tn_ps = ps.tile([128, 2 * D], mybir.dt.bfloat16, tag="tnps", name="tn_ps")
                tt_g = group_tiles[gi]
                qt_t = qt_ts[b]
                half = j % 2

                for c in range(DC):
                    stat = tt_g[:, c, goff * 128:(goff + 1) * 128]
                    if not dma_pair:
                        nc.tensor.transpose(
                            tn_ps[:, half * D + c * 128:half * D + (c + 1) * 128], stat, ident)
                    nc.tensor.matmul(st_g[:, j * P:(j + 1) * P], lhsT=stat, rhs=qt_t[:, c, :],
                                     start=(c == 0), stop=(c == DC - 1))
                half_accum.append((b, t))

                if half == 1 and not dma_pair:
                    tn_sb = tn_pool.tile([128, 2 * D + 2], mybir.dt.bfloat16, tag="tn", name="tn_sb")
                    nc.vector.tensor_copy(
                        tn_sb.rearrange("p (k x) -> p k x", k=2)[:, :, 0:D],
                        tn_ps.rearrange("p (k x) -> p k x", k=2),
                    )
                    pair_tiles[p] = tn_sb
                if half == 1:
                    tn_halves.append(pair_tiles.pop(p))

                if j == GPT - 1:
                    # group complete: one exp for all 8 tiles (consumption is a
                    # whole phase away, so a late pt cannot stall anything)
                    pt_h = pt_pool.tile([128, GPT * P], mybir.dt.bfloat16, tag="pt", name="pt_h")
                    nc.scalar.activation(out=pt_h, in_=st_g,
                                         func=mybir.ActivationFunctionType.Exp, scale=SCALE)
                    for k, (b2, t2) in enumerate(half_accum):
                        mm2_q.append((b2, t2, pt_h[:, k * P:(k + 1) * P],
                                      tn_halves[k // 2], k % 2))
                    tn_halves = []
                    half_accum = []

            # Phase 2: all mm2 pairs back-to-back; their semaphore checks hide
            # under the 213/107-col streams (measured clean at ~330ns/pair).
            while mm2_q:
                mm2 = mm2_q.pop(0)
                mm2a(*mm2)
                mm2b(*mm2)

    nc.compile()
    return nc


def _get_nc() -> bass.Bass:
    global _cached_nc
    if _cached_nc is None:
        _cached_nc = build_bass_program()
    return _cached_nc


def _make_in_maps(x_latent: np.ndarray, prompts_latent: np.ndarray):
    tn_h = np.ascontiguousarray(x_latent.astype(BF16))             # [16, N, D]
    tt_h = np.ascontiguousarray(tn_h.transpose(0, 2, 1))           # [16, D, N]
    qt_h = np.ascontiguousarray(prompts_latent.astype(BF16).transpose(0, 2, 1))
    idm = np.eye(128, dtype=BF16)
    return [
        {
            "qt": qt_h[c * BPC:(c + 1) * BPC],
            "tt": tt_h[c * BPC:(c + 1) * BPC],
            "tn": tn_h[c * BPC:(c + 1) * BPC],
            "idm": idm,
        }
        for c in range(N_CORES)
    ]


def run(x_latent: np.ndarray, prompts_latent: np.ndarray, trace: bool = False):
    """Run on all 8 cores; returns (output [16, 64, 768] f32, BassKernelResults)."""
    nc = _get_nc()
    in_maps = _make_in_maps(np.asarray(x_latent), np.asarray(prompts_latent))
    res = run_bass_kernel_spmd(nc, in_maps, list(range(N_CORES)), trace=trace)
    out = np.concatenate([np.asarray(r["out"]) for r in res.results], axis=0)
    return out.astype(np.float32), res


def kernel(x_latent: np.ndarray, prompts_latent: np.ndarray) -> np.ndarray:
    out, _ = run(x_latent, prompts_latent, trace=False)
    return out
